# revision 28
# baseline (speedup 1.0000x reference)
"""Trainium2 Bass kernel for nn_MultiHeadAttentionQuantum.

Math simplification (verified vs reference to ~5e-7):
  The per-token quantum feature map RX(x+theta) -> CNOT ring -> <Z_w>
  collapses to products of cosines. With u_w = cos(x_w + theta_w):
      q_0 = u1*u2*...*u7
      q_w = u0*u1*...*uw   (w = 1..7)
  Then per batch: scores = q @ q.T / sqrt(2); attn = softmax(scores);
  out = attn @ q; out' = swapaxes(out,1,2).reshape(S,8);  y = out' @ Wc.T + b.
  exp is computed with a folded bias -ln(64) (softmax shift-invariance), so
  all fp16 strip sums stay < 65504; row sums come free as a ones-column.

Sharding: data-parallel over batch: 16 batches -> 8 cores x 2 batches.

Per-core device pipeline (steady state is ACT-bound: exp of S^2 scores at
1 elem/lane/cycle @1.2GHz is the roofline; everything else hides under it):
  phase Q, pass 1 (both batches): DMA x p-major (token s = 32p+t), +theta
    +pi/2 per wire (DVE), range-reduce mod 2pi, ACT Sin (both Sins early so
    the exp table loads once before the attention stream).
  phase Q, pass 2 (both batches): 13 strided DVE muls -> q9 [128,T,9] fp32
    (col 8 = ones) + fp16 copy, PE-transpose chunks -> qT [128,S] fp16 with
    feature rows replicated at partition strips 0/32/64/96; PSUM->SBUF
    evacuation on ACT (idle here; "copy" is in every table set).
  phase A (per batch): flat slot list (i-block x 11 exp groups of JG=3
    j-chunks).  Scores run one slot ahead of exp (K=8 fp16 matmuls, row
    strips round-robin tj%4, 3 concurrent streams -> 3 PSUM banks).  One
    ACT exp instr per group (scale 1/sqrt2, bias -ln64, PSUM->SBUF fp16).
    Accums (col-group-packed X[32s:32s+9] += q9_j^T @ exp, strip s = tj%4)
    are deferred 2 slots and emitted in bunches of 3 groups so the PE gets
    ~4us contiguous busy runs (HAM clock-gate friendliness) and exp never
    waits on a jammed PE FIFO.
    normalize (after an i-block's last accum group): DVE fp16 copy X->Xs,
    4 fp16 sel matmuls (sum the 4 strips AND transpose to token-major; fp16
    single-pass avoids the fp32 LOW/HIGH double-pass that stalled the PE),
    DVE reciprocal of the ones-row, one broadcast multiply -> osb.
  phase C (per batch, 4 pieces, piece a = output tokens {32p+8a+j} = osb
    cols [64a,64a+64) = i-blocks {2a,2a+1}): per i-block, store its osb
    slice to token-ordered oscr rows and gather its half of glh (ones row 8
    for the bias); after normalize(2a+1), 8 fp16 matmuls vs wcb=[Wc.T;b]
    into spare cols of the live Y PSUM bank (serial drains), DVE copy, and
    a strided y store.  Everything overlaps the attention stream; only the
    last piece trails the final exp.
"""

import numpy as np

import concourse.bass as bass
import concourse.bacc as bacc
import concourse.tile as tile
from concourse import mybir
from concourse.masks import make_identity
from concourse._compat import with_exitstack

F32 = mybir.dt.float32
F16 = mybir.dt.float16
AF = mybir.ActivationFunctionType
P = 128
E = 8
E9 = 9
IB = 512          # i-block width (tokens per output accumulation block)
JG = 3            # j-chunks per exp group (3 PSUM banks per scores buffer)
INV_SQRT2 = 0.7071067811865476
LN64 = 4.1588830833596715  # exp bias: softmax shift keeps fp16 sums < 65504


@with_exitstack
def _body(ctx, tc, x_in, thp, wcb, sel, y, oscr, S, NB):
    nc = tc.nc
    T = S // P                 # token-chunks (tokens per partition)
    NIB = S // IB              # i-blocks per batch
    M4 = S // (P * E)          # row-tiles per combine feature block
    CPI = IB // P              # chunks per i-block (4)

    const = ctx.enter_context(tc.tile_pool(name="const", bufs=1))
    qpool = ctx.enter_context(tc.tile_pool(name="qdata", bufs=1))
    work = ctx.enter_context(tc.tile_pool(name="work", bufs=2))
    expp = ctx.enter_context(tc.tile_pool(name="expp", bufs=5))
    scps = ctx.enter_context(tc.tile_pool(name="scps", bufs=2, space="PSUM"))
    outps = ctx.enter_context(tc.tile_pool(name="outps", bufs=2, space="PSUM"))

    ident = const.tile([P, P], F32)
    make_identity(nc, ident[:])
    idh = const.tile([P, E9], F16)
    nc.vector.tensor_copy(idh[:], ident[:, 0:E9])
    thp_sb = const.tile([P, E], F32)
    nc.sync.dma_start(thp_sb[:], thp[:])
    wcb_sb = const.tile([P, E], F16)
    nc.sync.dma_start(wcb_sb[:], wcb[:])
    sel_sb = const.tile([P, E9], F16)
    nc.sync.dma_start(sel_sb[:], sel[:])
    bln = const.tile([P, 1], F32)
    nc.vector.memset(bln[:], -LN64)

    q9 = [qpool.tile([P, T * E9], F32, name=f"q9_{b}") for b in range(NB)]
    q9h = [qpool.tile([P, T * E9], F16, name=f"q9h_{b}") for b in range(NB)]
    qT = [qpool.tile([P, S], F16, name=f"qT_{b}") for b in range(NB)]
    osb = [qpool.tile([P, T * E], F16, name=f"osb_{b}") for b in range(NB)]

    # ---------------- phase Q: quantum features --------------------------
    # pass 1 (both batches): x DMA -> +theta -> range-reduce -> ACT Sin.
    # Doing both Sins up front lets the exp table load right after, before
    # the attention stream needs ACT.
    uss = []
    for b in range(NB):
        xb = x_in[b].rearrange("(p t) w -> p (t w)", p=P)
        xs = work.tile([P, T * E], F32, tag="xs")
        nc.sync.dma_start(xs[:], xb)
        x3 = xs.rearrange("p (t w) -> p t w", w=E)
        ph = work.tile([P, T * E], F32, tag="ph")
        p3 = ph.rearrange("p (t w) -> p t w", w=E)
        for w in range(E):
            nc.vector.tensor_scalar_add(p3[:, :, w], x3[:, :, w], thp_sb[:, w : w + 1])
        # range-reduce ph mod 2*pi into [-pi, pi] (Sin spline domain):
        # n = round(ph / 2pi) via the fp32 magic-constant trick, ph -= n * 2pi
        MAGIC = 12582912.0  # 1.5 * 2**23
        TWO_PI = 6.283185307179586
        rt = work.tile([P, T * E], F32, tag="rt")
        nc.vector.tensor_scalar(
            rt[:], ph[:], 1.0 / TWO_PI, MAGIC, mybir.AluOpType.mult, mybir.AluOpType.add
        )
        nc.vector.tensor_scalar(
            rt[:], rt[:], MAGIC, -TWO_PI, mybir.AluOpType.subtract, mybir.AluOpType.mult
        )
        nc.vector.tensor_add(ph[:], ph[:], rt[:])
        us = work.tile([P, T * E], F32, tag="us")
        nc.scalar.activation(us[:], ph[:], AF.Sin)
        uss.append(us)

    # pass 2 (both batches): cosine products -> q9/q9h -> qT transposes
    for b in range(NB):
        u3 = uss[b].rearrange("p (t w) -> p t w", w=E)
        q = q9[b]
        nc.vector.memset(q[:], 1.0)
        q3 = q.rearrange("p (t e) -> p t e", e=E9)
        nc.vector.tensor_mul(q3[:, :, 1], u3[:, :, 0], u3[:, :, 1])
        for w in range(2, E):
            nc.vector.tensor_mul(q3[:, :, w], q3[:, :, w - 1], u3[:, :, w])
        nc.vector.tensor_mul(q3[:, :, 0], u3[:, :, 1], u3[:, :, 2])
        for w in range(3, E):
            nc.vector.tensor_mul(q3[:, :, 0], q3[:, :, 0], u3[:, :, w])

        nc.vector.tensor_copy(q9h[b][:], q[:])
        # transpose q9 token-chunks into qT rows 0:9 (col 128*t + p), then
        # replicate the slice to partition strips 32/64/96 via SBUF DMA
        for c0 in range(0, T, 4):
            tp = scps.tile([P, IB], F32, tag="sc")
            for c in range(4):
                nc.tensor.transpose(
                    tp[0:E9, c * P : (c + 1) * P], q3[:, c0 + c, :], ident[:]
                )
            cols = slice(c0 * P, (c0 + 4) * P)
            # evacuate on ACT (idle during phase Q); copy is in every table set
            nc.scalar.copy(qT[b][0:E9, cols], tp[0:E9, :])
            for r in range(1, 4):
                nc.sync.dma_start(qT[b][32 * r : 32 * r + E, cols], qT[b][0:E, cols])

    # HAM warmup: one contiguous ~4us PE-busy burst of dummy matmuls flips
    # the clock gate to K=8/8 (2.4 GHz).  The steady-state micro-gaps after
    # are far below the ~3.4us MID idle window, so the PE stays warm.
    wu = outps.tile([P, IB], F32, tag="X")
    for _ in range(10):
        nc.tensor.matmul(
            wu[:], qT[0][0:E, 0:P], qT[0][0:E, 0:IB], start=True, stop=True
        )

    # ---------------- phases A + C, batch-pipelined -----------------------
    # phase C runs as 4 pieces per batch; piece a covers output tokens
    # {32p+8a+j} which live in osb cols [64a, 64a+64) = i-blocks {2a, 2a+1},
    # so each piece is emitted right after normalize(2a+1) and overlaps the
    # remaining attention stream.  Per piece: osb slice -> oscr (DRAM,
    # token-ordered rows 32p+8a+j), gather -> glh (lhsT layout, ones row 8
    # for the bias), 8 fp16 matmuls vs wcb into spare cols of the live Y
    # PSUM bank (serial drains), direct PSUM->DRAM y store.
    glh = qpool.tile([P, M4 * P * E], F16, name="glh")
    # ones fill on idle GPSIMD: keeps the 3.5us memset off the DVE startup
    # critical path (first use of glh is ~i-block 3).
    nc.gpsimd.memset(glh[:], 1.0)
    glh4 = glh.rearrange("p (a pp k) -> p a pp k", pp=P, k=E)

    def emit_piece_dma(b, ib):
        # store this i-block's osb slice to token-ordered oscr rows, then
        # gather its half of combine piece a = ib//2 into glh partitions
        # [4h, 4h+4) (h = ib%2); the other half arrives with the pair block.
        a, h = ib // 2, ib % 2
        ov = oscr[b].rearrange("(p i t) w -> p i (t w)", i=NIB, t=4)
        nc.sync.dma_start(ov[:, ib], osb[b][:, 32 * ib : 32 * ib + 32])
        gv = oscr[b].rearrange("(p a j) w -> a j p w", a=M4, j=E)
        nc.sync.dma_start(glh4[4 * h : 4 * h + 4, a], gv[a, 4 * h : 4 * h + 4])

    def emit_piece_mms(b, a, Y):
        for k in range(E):
            nc.tensor.matmul(
                Y[:, P + k * E : P + (k + 1) * E],
                glh4[0:E9, a, :, k],
                wcb_sb[0:E9, :],
                start=True,
                stop=True,
            )
        ystg = work.tile([P, E * E], F32, tag="ystg")
        nc.vector.tensor_copy(ystg[:], Y[:, P : P + E * E])
        yv = y[b].rearrange("(k p a) j -> p a k j", a=M4, p=P)
        nc.sync.dma_start(yv[:, a], ystg.rearrange("p (k j) -> p k j", j=E))

    for b in range(NB):
        qh3 = q9h[b].rearrange("p (t e) -> p t e", e=E9)
        o3 = osb[b].rearrange("p (t w) -> p t w", w=E)
        pending = None  # deferred normalize of the previous i-block

        def normalize(X, ib):
            Xs = work.tile([P, IB], F16, tag="Xs")
            nc.vector.tensor_copy(Xs[:], X[:])
            Y = outps.tile([P, IB], F32, tag="X")
            for c in range(CPI):
                nc.tensor.matmul(
                    Y[:, c * E9 : (c + 1) * E9],
                    Xs[:, c * P : (c + 1) * P],
                    sel_sb[:],
                    start=True,
                    stop=True,
                )
            Y3 = Y[:, 0 : CPI * E9].rearrange("p (c e) -> p c e", e=E9)
            rec = work.tile([P, CPI], F32, tag="rec")
            nc.vector.reciprocal(rec[:], Y3[:, :, 8])
            # one broadcast multiply for all 4 chunks (rec stride-0 over w)
            ya, ra = bass.broadcast_tensor_aps(
                Y3[:, :, 0:E], rec.rearrange("p (c o) -> p c o", o=1)
            )
            nc.vector.tensor_mul(o3[:, ib * CPI : (ib + 1) * CPI, :], ya, ra)
            emit_piece_dma(b, ib)
            return Y

        def emit_accums(Xa, g0a, gna, exa):
            for g in range(gna):
                tj = g0a + g
                cs = 32 * (tj % 4)
                nc.tensor.matmul(
                    Xa[cs : cs + E9, :],
                    qh3[:, tj, :],
                    exa[:, g * IB : (g + 1) * IB],
                    start=(tj == 0),
                    stop=(tj == T - 1),
                    tile_position=(0, cs),
                    skip_group_check=True,
                )

        # Bunched pipeline over flat (ib, g0) slots: scores run one slot
        # ahead of exp; accums are deferred 2 slots and emitted in bunches of
        # BUNCH groups so the PE sees ~4us contiguous busy runs.  Fragmented
        # ~1us runs can never re-warm the HAM clock-gate (needs a full 3.4us
        # busy window for K=8/8); bunches can, and the inter-bunch idle stays
        # far below the MID re-throttle window, so the PE runs at 2.4 GHz.
        slots = [
            (ib, g0, min(JG, T - g0))
            for ib in range(NIB)
            for g0 in range(0, T, JG)
        ]
        BUNCH = 3
        bunch = []  # (X, g0, gn, ex, last_of_ib, ib)

        def emit_exp(h):
            sc, Xc, g0, gn, ib, _ = h
            ex = expp.tile([P, JG * IB], F16, tag="ex")
            nc.scalar.activation(
                ex[:, 0 : gn * IB], sc[:, 0 : gn * IB], AF.Exp,
                scale=INV_SQRT2, bias=bln[:, 0:1],
            )
            h[5] = ex

        def flush_bunch():
            done = []
            for Xa, g0a, gna, exa, lastg, iba in bunch:
                emit_accums(Xa, g0a, gna, exa)
                if lastg:
                    done.append((Xa, iba))
            bunch.clear()
            for Xa, iba in done:
                Yn = normalize(Xa, iba)
                if iba % 2 == 1:
                    emit_piece_mms(b, iba // 2, Yn)

        Xcur = None
        hist = []
        for i, (ib, g0, gn) in enumerate(slots):
            if g0 == 0:
                Xcur = outps.tile([P, IB], F32, tag="X")
                nc.vector.memset(Xcur[:], 0.0)
            sc = scps.tile([P, JG * IB], F32, tag="sc")
            for g in range(gn):
                tj = g0 + g
                rb = 32 * (tj % 4)
                nc.tensor.matmul(
                    sc[:, g * IB : (g + 1) * IB],
                    qT[b][rb : rb + E, tj * P : (tj + 1) * P],
                    qT[b][rb : rb + E, ib * IB : (ib + 1) * IB],
                    start=True,
                    stop=True,
                    tile_position=(rb, 0),
                )
            hist.append([sc, Xcur, g0, gn, ib, None])
            if i >= 1:
                emit_exp(hist[i - 1])
            if i >= 2:
                h = hist[i - 2]
                bunch.append((h[1], h[2], h[3], h[5], h[2] + JG >= T, h[4]))
                hist[i - 2] = None
                if len(bunch) >= BUNCH:
                    flush_bunch()
        # drain: last exp, remaining accum groups, final normalize + piece
        n = len(slots)
        emit_exp(hist[n - 1])
        for j in (n - 2, n - 1):
            h = hist[j]
            bunch.append((h[1], h[2], h[3], h[5], h[2] + JG >= T, h[4]))
        flush_bunch()

def build_nc(S=4096, NB=2):
    nc = bacc.Bacc(None, target_bir_lowering=False)
    x_in = nc.dram_tensor("x", (NB, S, E), F32, kind="ExternalInput")
    thp = nc.dram_tensor("thp", (P, E), F32, kind="ExternalInput")
    wcb = nc.dram_tensor("wcb", (P, E), F16, kind="ExternalInput")
    sel = nc.dram_tensor("sel", (P, E9), F16, kind="ExternalInput")
    y = nc.dram_tensor("y", (NB, S, E), F32, kind="ExternalOutput")
    oscr = nc.dram_tensor("oscr", (NB, S, E), F16)
    with tile.TileContext(nc) as tc:
        _body(tc, x_in[:], thp[:], wcb[:], sel[:], y[:], oscr[:], S, NB)
    nc.compile()
    return nc


def host_inputs(theta, w_combine, b_combine):
    thp = np.tile(
        (np.asarray(theta, np.float32) + np.float32(np.pi / 2))[None, :], (P, 1)
    ).astype(np.float32)
    wcb9 = np.concatenate(
        [np.asarray(w_combine, np.float32).T, np.asarray(b_combine, np.float32)[None]],
        axis=0,
    ).astype(np.float32)
    wcb = np.zeros((P, E), np.float16)
    for st in range(4):
        wcb[32 * st : 32 * st + E9] = wcb9.astype(np.float16)
    sel = np.zeros((P, E9), np.float16)
    for st in range(4):
        for e in range(E9):
            sel[32 * st + e, e] = 1.0
    return thp, wcb, sel


_NC_CACHE = {}


def kernel(x, theta, w_combine, b_combine):
    from concourse.bass_utils import run_bass_kernel_spmd

    x = np.asarray(x, np.float32)
    B, S, _ = x.shape
    NCORES = 8
    NB = B // NCORES
    key = (S, NB)
    if key not in _NC_CACHE:
        _NC_CACHE[key] = build_nc(S=S, NB=NB)
    nc = _NC_CACHE[key]
    thp, wcb, sel = host_inputs(theta, w_combine, b_combine)
    in_maps = [
        {"x": x[c * NB : (c + 1) * NB], "thp": thp, "wcb": wcb, "sel": sel}
        for c in range(NCORES)
    ]
    res = run_bass_kernel_spmd(nc, in_maps, list(range(NCORES))).results
    return np.concatenate([res[c]["y"] for c in range(NCORES)], axis=0)

